# revision 37
# baseline (speedup 1.0000x reference)
"""BrahmaAttention (GQA prefill with KV cache) on 8 Trainium2 NeuronCores.

Problem: B=4, S=1024, C=1024 (cache), H=16 q-heads, G=4 kv-heads, D=128.
    q = hs @ wq.T ; k = hs @ wk.T ; v = hs @ wv.T
    rope(q, k) (interleaved pairs, positions C..C+S)
    k_full/v_full = concat(cache, new)           # K = 2048 keys
    out = softmax(q k^T / sqrt(D)) @ v_full @ wo.T
(attention_mask is all-zeros by construction - full attention, no masking.)

Sharding: 4-way data parallel over batch x 2-way tensor parallel over heads.
core (b, hg) handles batch b, q-heads hg*8..hg*8+8, kv-heads hg*2..hg*2+2 and
computes a partial output projection over its 1024 hidden columns; the host
sums the two partials per batch (the TP all-reduce done on host at gather).

Host-side prep folded into the shards:
  - 1/sqrt(D) folded into wq.
  - RoPE even/odd interleave permuted to [evens|odds] via wq/wk row
    permutation and cache_k last-dim permutation, so on-chip RoPE is
    half-tile elementwise ops (partitions 0-63 = even, 64-127 = odd lanes).
  - All tensors bf16 (incl. qT/kT and the y output partials: HW runs f32r
    matmuls ~17% slower than bf16, and bf16 halves the DMA); weights are
    packed partition-major in DRAM so every DMA line is >=4KB (256B lines
    run below SDMA line-rate).

Engine balance: the PE streams 1 row/cycle for bf16, so all matmuls are
minimal-row; exp runs only on ACT (~8.5us per head-block vs ~7us of PE
score+AV work per block), so the kernel interleaves:
  - Q-projections for heads 2..7 into the first attention half,
  - the first half's output projection into the second attention half,
so the PE always has projection work to fill ACT-gated gaps.  The K/Q01
projections are emitted k-major against the arriving hsT DMA stream
(small ~36-matmul warm-up keeps the PE p-state ramp alive), y-stores ride
the ACT HWDGE queue so they never head-block the SP input queue, and the
softmax denominator is one DVE add per chunk-group into a [P,2,SH]
accumulator, folded and cross-partition-summed by a single PE ones-matmul
(the Pool partition_all_reduce is ~7us ucode on real HW and sat on every
block's critical path), inverted with the fast DVE reciprocal, and the
normalization is fused into the PSUM->SBUF copy of the AV output.
"""

import numpy as np

B, S, C, H, G, D = 4, 1024, 1024, 16, 4, 128
HID = H * D
P = 128
NH, NG = 8, 2          # per-core q heads / kv heads
KC = (C + S) // P      # 16 key chunks
KT = 16                # hid contraction tiles
SH = 512               # s-half (PSUM bank free size)
N_CORES = 8

_PERM = np.concatenate([np.arange(0, D, 2), np.arange(1, D, 2)])

_BUILT = {}
_FILLER = True  # light per-cg filler: just enough PE work to cover ACT's exp deficit
_WARMUP = 36    # f32r ones-matmuls (~213ns ea) covering the wk+hsT[0] DMA window


def _mm(nc, out, lhsT, rhs, **kw):
    nc.tensor.matmul(out, lhsT, rhs, **kw)


def _rope_half(nc, pool, f32, psum_in, out_ap, cs_cc, cs_pm, mult, add,
               act_swap=False):
    """out = psum_in*[cos;cos] + swap_partition_halves(psum_in*[sin;-sin]).

    psum_in is a raw projected [128, SH] tile with evens on partitions 0-63
    and odds on 64-127; out gets the roped value in the same layout.
    The partition-half swap runs on ACT during the projection prologue
    (ACT idle there) and on Pool during the attention sections (ACT is
    saturated with exps there).
    """
    a = pool.tile([P, SH], f32, tag="ropeA", name="ropeA")
    b = pool.tile([P, SH], f32, tag="ropeB", name="ropeB")
    s = pool.tile([P, SH], f32, tag="ropeS", name="ropeS")
    nc.vector.tensor_tensor(a[:], psum_in[:], cs_cc[:], mult)
    nc.vector.tensor_tensor(b[:], psum_in[:], cs_pm[:], mult)
    if act_swap:
        nc.scalar.copy(s[0:64, :], b[64:128, :])
        nc.scalar.copy(s[64:128, :], b[0:64, :])
    else:
        nc.gpsimd.tensor_copy(s[0:64, :], b[64:128, :])
        nc.gpsimd.tensor_copy(s[64:128, :], b[0:64, :])
    nc.vector.tensor_tensor(out_ap, a[:], s[:], add)


def build_bass(unroll=1):
    """Build + compile the per-core Bass program (identical on all cores)."""
    if unroll in _BUILT:
        return _BUILT[unroll]

    import concourse.mybir as mybir
    import concourse.tile as tile
    from concourse import bacc

    f32 = mybir.dt.float32
    f32r = mybir.dt.float32r
    bf16 = mybir.dt.bfloat16
    mult = mybir.AluOpType.mult
    add = mybir.AluOpType.add
    Exp = mybir.ActivationFunctionType.Exp

    nc = bacc.Bacc("TRN2", target_bir_lowering=False, debug=False)

    # weight layouts are partition-major in DRAM (contiguous KT*P per
    # partition) so every DMA line is >=4KB -- 256B lines run below SDMA
    # line-rate (read-modify-write under 512B).
    hsT_d = nc.dram_tensor("hsT", [KT, P, S], bf16, kind="ExternalInput")
    wq_d = nc.dram_tensor("wqT", [NH, P, KT * P], bf16, kind="ExternalInput")
    wk_d = nc.dram_tensor("wkT", [NG, P, KT * P], bf16, kind="ExternalInput")
    wv_d = nc.dram_tensor("wvT", [P, KT, NG * P], bf16, kind="ExternalInput")
    wo_d = nc.dram_tensor("woT", [NH, P, HID], bf16, kind="ExternalInput")
    ck_d = nc.dram_tensor("ckT", [NG, P, C], bf16, kind="ExternalInput")
    cv_d = nc.dram_tensor("cvP", [P, C // P, NG * P], bf16,
                          kind="ExternalInput")
    cc_d = nc.dram_tensor("cs_cc", [P, S], bf16, kind="ExternalInput")
    pm_d = nc.dram_tensor("cs_pm", [P, S], bf16, kind="ExternalInput")
    y_d = nc.dram_tensor("y", [S, HID], bf16, kind="ExternalOutput")

    with tile.TileContext(nc) as tc:
        with tc.tile_pool(name="const", bufs=1) as const:
            ones_f = const.tile([P, P], f32, name="ones_f")
            nc.any.memset(ones_f[:], 1.0)
            ones128 = const.tile([P, P], f32r, name="ones128")
            nc.vector.tensor_copy(ones128[:], ones_f[:])
            ones_bf = const.tile([P, P], bf16, name="ones_bf")
            nc.vector.tensor_copy(ones_bf[:], ones_f[:])
            cs_cc = const.tile([P, S], bf16, name="cs_cc")
            cs_pm = const.tile([P, S], bf16, name="cs_pm")

            with tc.tile_pool(name="wk_pool", bufs=2) as wk_pool:
                wk_pre = None
                for it in range(unroll):
                    wk_pre = _emit_iteration(
                        nc, tc, f32, f32r, bf16, mult, add, Exp,
                        hsT_d, wq_d, wk_d, wv_d, wo_d, ck_d, cv_d, y_d,
                        ones128, ones_bf, cs_cc, cs_pm, wk_pool,
                        cs_load=(cc_d, pm_d) if it == 0 else None,
                        wk_pre=wk_pre,
                        prefetch_next=(it + 1 < unroll),
                    )

    nc.compile()
    _BUILT[unroll] = nc
    return nc


def _emit_iteration(nc, tc, f32, f32r, bf16, mult, add, Exp,
                    hsT_d, wq_d, wk_d, wv_d, wo_d, ck_d, cv_d, y_d,
                    ones128, ones_bf, cs_cc, cs_pm, wk_pool, cs_load=None,
                    wk_pre=None, prefetch_next=False):
    with (
        tc.tile_pool(name="persist", bufs=1) as persist,
        tc.tile_pool(name="proj", bufs=1) as proj_pool,
        tc.tile_pool(name="wq_pool", bufs=2) as wq_pool,
        tc.tile_pool(name="rope", bufs=1) as rope_pool,
        tc.tile_pool(name="attn_sb", bufs=1) as attn_pool,
        tc.tile_pool(name="probs", bufs=1) as probs_pool,
        tc.tile_pool(name="wo_pool", bufs=4) as wo_pool,
        tc.tile_pool(name="small", bufs=2) as small_pool,
        tc.tile_pool(name="ps", bufs=1, space="PSUM") as ps,
    ):
        qT = persist.tile([P, NH, S], bf16, name="qT")
        kT = persist.tile([P, NG, C + S], bf16, name="kT")
        vF = persist.tile([P, KC, NG * P], bf16, name="vF")
        attn = attn_pool.tile([P, NH, S], bf16, name="attn_sb")
        hsT = proj_pool.tile([P, KT, S], bf16, name="hsT_sb")
        wv = proj_pool.tile([P, KT, NG * P], bf16, name="wv_sb")

        # ---------------- DMA queue (ordered by first use) ----------------
        if cs_load is not None:
            # PE/HAM warm-up (iteration 0 only): enough back-to-back tiny
            # matmuls to keep the p-state ramp alive until wk + the first
            # hsT chunk land; the k-major K-projection then paces the PE
            # against the arriving hsT stream.
            pw = ps.tile([P, SH], f32, tag="av", bufs=2, name="pwarm")
            for i in range(_WARMUP):
                _mm(nc, pw[:, 0:P], ones128[:], ones128[:],
                    start=(i == 0), stop=(i == _WARMUP - 1),
                    skip_group_check=True)
            wsink = small_pool.tile([1, 1], f32, tag="wsink", name="wsink")
            nc.vector.tensor_copy(wsink[:], pw[0:1, 0:1])
        def load_wks():
            tiles = []
            for g in range(NG):
                wk = wk_pool.tile([P, KT, P], bf16, tag="wk", name="wk_sb")
                nc.sync.dma_start(
                    wk[:], wk_d[g].rearrange("p (k m) -> p k m", k=KT))
                tiles.append(wk)
            return tiles

        wks = wk_pre if wk_pre is not None else load_wks()
        for i in range(8):
            nc.sync.dma_start(
                hsT[:, 2 * i:2 * i + 2, :],
                hsT_d[2 * i:2 * i + 2].rearrange("k p s -> p k s"),
            )
        if cs_load is not None:
            nc.sync.dma_start(cs_cc[:], cs_load[0][:])
            nc.sync.dma_start(cs_pm[:], cs_load[1][:])
        nc.sync.dma_start(wv[:], wv_d[:])
        wqs = {}
        for h in range(2):
            wq = wq_pool.tile([P, KT, P], bf16, tag="wq", name="wq_sb")
            nc.sync.dma_start(
                wq[:], wq_d[h].rearrange("p (k m) -> p k m", k=KT))
            wqs[h] = wq
        for g in range(NG):
            nc.sync.dma_start(kT[:, g, 0:C], ck_d[g])
        nc.sync.dma_start(vF[:, 0:C // P, :], cv_d[:])

        # ---------------- projection helper (one [P, SH] half at a time) ---
        # prologue projections alternate between the psA and av PSUM rings
        # (attention rings are idle before section 4) so the rope chain of
        # one half never blocks the next half's matmuls.
        _ppick = [0]

        def emit_proj(w, dst_fn):
            for half in range(2):
                hsl = slice(half * SH, (half + 1) * SH)
                tag = ("av", "psA")[_ppick[0] % 2]
                _ppick[0] += 1
                pp = ps.tile([P, SH], f32, tag=tag, bufs=2, name="pp")
                for k in range(KT):
                    _mm(nc, pp[:], w[:, k, :], hsT[:, k, hsl],
                        start=(k == 0), stop=(k == KT - 1))
                _rope_half(nc, rope_pool, f32, pp, dst_fn(hsl),
                           cs_cc[:, hsl], cs_pm[:, hsl], mult, add,
                           act_swap=True)

        # ---- section 1: K-projection + rope (new keys -> kT[:, g, C:]) ----
        # k-major across all 4 (g, half) accumulators so the PE consumes
        # each hsT chunk as its DMA lands instead of waiting for the full
        # tensor; the 4 halves live in the 2 score-ring PSUM tiles (idle
        # until section 4).
        pk = [ps.tile([P, 2, SH], f32, tag="score", bufs=2, name="pk")
              for _ in range(NG)]
        for k in range(KT):
            for g in range(NG):
                for half in range(2):
                    _mm(nc, pk[g][:, half, :], wks[g][:, k, :],
                        hsT[:, k, half * SH:(half + 1) * SH],
                        start=(k == 0), stop=(k == KT - 1),
                        skip_group_check=True)
        for g in range(NG):
            for half in range(2):
                hsl = slice(half * SH, (half + 1) * SH)
                _rope_half(nc, rope_pool, f32, pk[g][:, half, :],
                           kT[:, g, C + hsl.start:C + hsl.stop],
                           cs_cc[:, hsl], cs_pm[:, hsl], mult, add,
                           act_swap=True)
        # ---- section 2: V-projection (tokens on partitions) ----
        # V runs BEFORE Q01 so its DVE copybacks (and the av/psA ring
        # slots they release) retire under Q01's matmuls -- the first
        # attention block then starts with a free av ring instead of the
        # ~3us PE stall TimelineSim showed at the V->attention boundary.
        for mv in range(S // P // 2):
            pv = ps.tile([P, SH], f32, tag=("av", "psA")[mv % 2], bufs=2,
                         name="pv")
            for m2 in range(2):
                m = 2 * mv + m2
                vsl = slice(m2 * NG * P, (m2 + 1) * NG * P)
                for k in range(KT):
                    _mm(nc, pv[:, vsl], hsT[:, k, m * P:(m + 1) * P],
                        wv[:, k, :], start=(k == 0), stop=(k == KT - 1))
            nc.vector.tensor_copy(
                vF[:, C // P + 2 * mv:C // P + 2 * mv + 2, :], pv[:])
        # ---- section 3: Q-projection heads 0, 1 ----
        for h in range(2):
            emit_proj(wqs[h], lambda hsl, h=h: qT[:, h, hsl])

        # ---------------- attention head-block ----------------
        # filler: iterator that emits one PE filler matmul per next() --
        # keeps the PE fed while ACT works through the exps (the attention
        # inner loop alone is ACT-bound: ~8.6us of exp per head-block vs
        # ~7us of score+AV matmuls).
        def attn_head(sh, h, filler=None, fpc=0):
            ssl = slice(sh * SH, (sh + 1) * SH)
            g = h // (NH // NG)
            NCG = KC // 2  # chunk groups of 2
            probs = [None] * NCG
            pav = ps.tile([P, SH], f32, tag="av", bufs=2, name="pav")
            accA = small_pool.tile([P, 2, SH], bf16, tag="accA", name="accA")

            def emit_scores(cg):
                pssc = ps.tile([P, 2, SH], f32, tag="score", bufs=2,
                               name="pscore")
                for j in range(2):
                    c = cg * 2 + j
                    _mm(nc, pssc[:, j, :], kT[:, g, c * P:(c + 1) * P],
                        qT[:, h, ssl], start=True, stop=True)
                # ring depth 10 (> the 8 chunk-groups of one block) so the
                # next block's first exp never waits on this block's
                # same-slot AV/DVE readers if HW timing skews late
                pt = probs_pool.tile([P, 2, SH], bf16, tag="probs",
                                     bufs=10, name="probs_t")
                nc.scalar.activation(pt[:], pssc[:], Exp)
                probs[cg] = pt

            def emit_av(cg):
                for j in range(2):
                    c = cg * 2 + j
                    _mm(nc, pav[:], vF[:, c, g * P:(g + 1) * P],
                        probs[cg][:, j, :],
                        start=(c == 0), stop=(c == KC - 1),
                        skip_group_check=True)

            # software pipeline: scores run 2 groups ahead of AV; the
            # denominator partial sums accumulate on DVE as full [P,2,SH]
            # tiles land (one add per chunk-group, not one per sub-chunk:
            # per-instruction overhead on HW makes fewer/wider DVE ops win).
            for cg in range(NCG + 2):
                if cg < NCG:
                    emit_scores(cg)
                if cg == 1:
                    nc.vector.tensor_tensor(
                        accA[:], probs[0][:], probs[1][:], add)
                elif 2 <= cg < NCG:
                    nc.vector.tensor_tensor(
                        accA[:], accA[:], probs[cg][:], add)
                if cg >= 2:
                    emit_av(cg - 2)
                if filler is not None:
                    for _ in range(fpc):
                        next(filler, None)

            # denominator: fold sub-chunk halves, cross-partition sum via a
            # PE ones-matmul (reusing an av-ring bank). The Pool
            # partition_all_reduce is slow ucode on real HW (~7us) and sat
            # on every block's critical path; the PE matmul costs 213ns.
            accC = small_pool.tile([P, SH], bf16, tag="accC", name="accC")
            nc.vector.tensor_tensor(accC[:], accA[:, 0, :], accA[:, 1, :],
                                    add)
            rcp = small_pool.tile([P, SH], f32, tag="rcp", name="rcp")
            pden = ps.tile([P, SH], f32, tag="av", bufs=2, name="pden")
            _mm(nc, pden[:], ones_bf[:], accC[:], start=True, stop=True)
            nc.vector.reciprocal_approx_fast(out=rcp[:], in_=pden[:])
            # normalized attention output (transposed), fused copyback
            nc.vector.tensor_tensor(attn[:, h, ssl], pav[:], rcp[:], mult)
            if filler is not None:
                for op in filler:
                    pass

        def proj_filler(w, hq):
            for half in range(2):
                hsl = slice(half * SH, (half + 1) * SH)
                pp = ps.tile([P, SH], f32, tag="psA", bufs=2, name="pp")
                for k in range(KT):
                    _mm(nc, pp[:], w[:, k, :], hsT[:, k, hsl],
                        start=(k == 0), stop=(k == KT - 1))
                    yield
                # act_swap: ACT has ~2us/block of slack under the exps,
                # while gpsimd copies are Pool ucode with ~us-scale fixed
                # cost on real HW.
                _rope_half(nc, rope_pool, f32, pp, qT[:, hq, hsl],
                           cs_cc[:, hsl], cs_pm[:, hsl], mult, add,
                           act_swap=True)

        # ---------------- output-projection group ----------------
        def wo_group(sh, gi, wons, tag="psA", ysb_act=False):
            n, mt = gi // 4, gi % 4
            m = sh * 4 + mt
            py = ps.tile([P, SH], f32, tag=tag, bufs=2, name="py")
            for h in range(NH):
                _mm(nc, py[:], attn[:, h, m * P:(m + 1) * P],
                    wons[n][:, h, :], start=(h == 0), stop=(h == NH - 1))
                yield
            ysb = small_pool.tile([P, SH], bf16, tag="ysb", bufs=3,
                                  name="ysb")
            if ysb_act:
                nc.scalar.copy(ysb[:], py[:])
            else:
                nc.vector.tensor_copy(ysb[:], py[:])
            # y-stores ride the ACT HWDGE queue so they never head-block
            # the next iteration's input loads on the SP queue.
            nc.scalar.dma_start(
                y_d[m * P:(m + 1) * P, n * SH:(n + 1) * SH], ysb[:])

        def chain(*gens):
            for gen in gens:
                for op in gen:
                    yield

        # ---- section 4: sh0 attention, Q-proj h2..7 interleaved;
        #      wo weights (shared by both halves) prefetch during it ----
        # section-4 weight loads ride the ACT HWDGE queue: they overlap the
        # SP queue's (next iteration's) input stream instead of serializing
        # behind it.
        wons = []
        for n in range(HID // SH):
            won = wo_pool.tile([P, NH, SH], bf16, tag="won", name="won")
            nc.scalar.dma_start(
                won[:],
                wo_d[:, :, n * SH:(n + 1) * SH].rearrange("h p n -> p h n"),
            )
            wons.append(won)
        # wq tiles prefetched one head ahead of their projection
        wq_tiles = {}

        def wq_prefetch(hq):
            wq = wq_pool.tile([P, KT, P], bf16, tag="wq", name="wq_sb")
            nc.scalar.dma_start(
                wq[:], wq_d[hq].rearrange("p (k m) -> p k m", k=KT))
            wq_tiles[hq] = wq

        wq_prefetch(2)
        for h in range(NH):
            if h + 3 < NH:
                wq_prefetch(h + 3)
            filler = proj_filler(wq_tiles[h + 2], h + 2) if h < NH - 2 else None
            if _FILLER:
                attn_head(0, h, filler, 2)
            else:
                attn_head(0, h)
                if filler is not None:
                    for op in filler:
                        pass

        # ---- section 5: sh1 attention, wo(sh0) groups interleaved ----
        for h in range(NH):
            filler = chain(wo_group(0, 2 * h, wons),
                           wo_group(0, 2 * h + 1, wons))
            if _FILLER:
                attn_head(1, h, filler, 1)
            else:
                attn_head(1, h)
                for op in filler:
                    pass

        # prefetch the NEXT iteration's K-weights now: their DMA configs
        # land on the SP queue ahead of section 6's y-stores, so the next
        # iteration's first matmul never waits on DMA at the boundary
        next_wks = load_wks() if prefetch_next else None

        # ---- section 6: wo(sh1); alternate PSUM tags (score/av rings are
        #      idle now) so consecutive groups never wait on a copyback ----
        for gi in range(16):
            for op in wo_group(1, gi, wons, tag="psA", ysb_act=True):
                pass
        return next_wks


def prep_inputs(hidden_states, freqs_cos, freqs_sin, cache_k, cache_v,
                wq, wk, wv, wo):
    """Shard + pre-transpose the full inputs into 8 per-core input maps."""
    import ml_dtypes
    bf = ml_dtypes.bfloat16
    f = np.float32
    scale = np.float32(1.0 / np.sqrt(D))
    wq_p = (wq.astype(f).reshape(H, D, HID)[:, _PERM, :] * scale)
    wk_p = wk.astype(f).reshape(G, D, HID)[:, _PERM, :]
    wv_r = wv.astype(f).reshape(G, D, HID)

    cc = freqs_cos.astype(f).T          # [64, S]
    ss = freqs_sin.astype(f).T
    cs_cc = np.ascontiguousarray(np.concatenate([cc, cc], axis=0)).astype(bf)
    cs_pm = np.ascontiguousarray(np.concatenate([ss, -ss], axis=0)).astype(bf)

    in_maps = []
    for b in range(B):
        hsT = np.ascontiguousarray(
            hidden_states[b].astype(f).T.reshape(KT, P, S)).astype(bf)
        for hg in range(2):
            hs_q = slice(hg * NH, (hg + 1) * NH)
            hs_kv = slice(hg * NG, (hg + 1) * NG)
            wqT = wq_p[hs_q].reshape(NH * D, HID).T          # [HID, 1024]
            wqT_t = np.ascontiguousarray(
                wqT.reshape(KT, P, NH, P).transpose(2, 1, 0, 3)
                .reshape(NH, P, KT * P)).astype(bf)
            wkT = wk_p[hs_kv].reshape(NG * D, HID).T         # [HID, 256]
            wkT_t = np.ascontiguousarray(
                wkT.reshape(KT, P, NG, P).transpose(2, 1, 0, 3)
                .reshape(NG, P, KT * P)).astype(bf)
            wvT = wv_r[hs_kv].reshape(NG * D, HID).T         # [HID, 256]
            wvT_t = np.ascontiguousarray(
                wvT.reshape(KT, P, NG * P).transpose(1, 0, 2)).astype(bf)
            woT = np.ascontiguousarray(
                wo.astype(f)[:, hg * NH * D:(hg + 1) * NH * D].T
                .reshape(NH, P, HID)).astype(bf)
            ckT = np.ascontiguousarray(
                cache_k[b].astype(f)[:, hs_kv][:, :, _PERM]
                .transpose(1, 2, 0)).astype(bf)
            cvP = np.ascontiguousarray(
                cache_v[b].astype(f)[:, hs_kv]
                .reshape(C // P, P, NG * P).transpose(1, 0, 2)).astype(bf)
            in_maps.append({
                "hsT": hsT, "wqT": wqT_t, "wkT": wkT_t, "wvT": wvT_t,
                "woT": woT, "ckT": ckT, "cvP": cvP,
                "cs_cc": cs_cc, "cs_pm": cs_pm,
            })
    return in_maps


def gather_output(results):
    """Sum the 2 TP partials per batch -> full [B, S, HID] output."""
    out = np.empty((B, S, HID), np.float32)
    for b in range(B):
        out[b] = (results[2 * b]["y"].astype(np.float32)
                  + results[2 * b + 1]["y"].astype(np.float32))
    return out


def kernel(hidden_states, freqs_cos, freqs_sin, attention_mask,
           cache_k, cache_v, wq, wk, wv, wo):
    # attention_mask is all-zeros by construction (see spec) - unused.
    from concourse.bass_utils import run_bass_kernel_spmd

    nc = build_bass(unroll=1)
    in_maps = prep_inputs(
        np.asarray(hidden_states), np.asarray(freqs_cos), np.asarray(freqs_sin),
        np.asarray(cache_k), np.asarray(cache_v),
        np.asarray(wq), np.asarray(wk), np.asarray(wv), np.asarray(wo))
    res = run_bass_kernel_spmd(nc, in_maps, core_ids=list(range(N_CORES)))
    return gather_output(res.results)

